# revision 12
# baseline (speedup 1.0000x reference)
"""CTC loss (focal-reweighted) Trainium2 Bass kernel, v3.

Data parallel over batch (128 examples -> 8 cores x 16). Per core:
  - host permutes the class axis: the <=401 classes that the CTC DP ever
    reads (blank + every label of the core's 16 examples) are packed into
    a 512-column tensor xg; the remaining 6113 columns form xr. Z (the
    softmax denominator) = rowsum exp(xg) + rowsum exp(xr), all on device.
  - xg ([128, 20*512], tile j = 16 examples x 8 timesteps, partition
    p = 16g + 2*tf + c -> example 2g+c, t = 8j+tf) loads in ONE DMA up
    front; 20 small exps (accum -> Z) feed 20 ap_gathers + parity
    reshuffles, so the CTC DP runs ~100us ahead of the big stream.
  - xr streams as 20 tiles of [128, 6113]; exp+accum -> Z columns. The
    last tile is split into 3 column chunks so the final exp (Z-only) is
    ~2us instead of ~6us after the stream ends.
  - CTC forward DP in prob space, even/odd state split: even (blank)
    states all share the per-example blank prob, so tracking
    u[i] = E[i] + O[i-1] and O gives 3 DVE ops per timestep:
      O' = (O + u) * eO;  u' = u * eb + shift(O')
    with renorms every 32 steps folded into the next step's multiplies
    and renorm sums taken from scalar_tensor_tensor's accum_out.
  - device exports raw u_T [16,26], S [16,5], Z [128,42]; host does all
    logs / readout / focal / mean in float64. v = u_T[len] directly.
Device DP assumes no consecutive repeated labels (skip always allowed on
odd states); host detects repeat examples and recomputes those exactly.
"""

import numpy as np

import concourse.bass as bass
import concourse.bacc as bacc
import concourse.tile as tile
from concourse import mybir
from concourse import bass_utils

B, T, C, L = 128, 160, 6625, 25
NCORES = 8
BL = B // NCORES          # 16 examples per core
NT = 20                   # t-blocks of 8 timesteps
G = 512                   # gathered-class columns (>= 401 needed)
CR = C - G                # 6113 remaining columns
W = L + 1                 # 26: [blank, 25 labels] per example per t
NZ = NT + (NT - 1) + 4    # Z accum columns: 20 xg + 19 xr + 4 last-tile
RENORM_ACC = (32, 64, 96, 128, 159)
RENORM_DIV = {33: 0, 65: 1, 97: 2, 129: 3}
LAST_CUTS = (0, 2038, 4076, 5676, CR)

F32 = mybir.dt.float32
I16 = mybir.dt.int16
LN2 = 0.6931471805599453
EXP_BIAS = -2.0 * LN2     # exp(x - 2 ln2) = 0.25 e^x, keeps DP in fp32 range


def _build_kernel():
    nc = bacc.Bacc("TRN2", target_bir_lowering=False, debug=False)
    xg = nc.dram_tensor("xg", [128, NT * G], F32, kind="ExternalInput").ap()
    xr = nc.dram_tensor("xr", [BL * T, CR], F32, kind="ExternalInput").ap()
    gidx = nc.dram_tensor("gidx", [128, 4], I16, kind="ExternalInput").ap()
    u_out = nc.dram_tensor("u_out", [BL, W], F32, kind="ExternalOutput").ap()
    s_out = nc.dram_tensor("s_out", [BL, 5], F32, kind="ExternalOutput").ap()
    z_out = nc.dram_tensor("z_out", [128, NZ], F32, kind="ExternalOutput").ap()

    with tile.TileContext(nc) as tc:
        with (
            tc.tile_pool(name="xio", bufs=5) as xio,
            tc.tile_pool(name="gap", bufs=4) as gap,
            tc.tile_pool(name="small", bufs=1) as small,
        ):
            gidx_sb = small.tile([128, 4], I16)
            nc.sync.dma_start(out=gidx_sb[:, :], in_=gidx[:, :])
            bias_sb = small.tile([128, 1], F32)
            nc.vector.memset(bias_sb[:, :], EXP_BIAS)

            Z = small.tile([128, NZ], F32)
            # gathered-class columns: one up-front DMA, then small exps,
            # gathers and reshuffles run far ahead of the big stream
            xg_sb = small.tile([128, NT * G], F32)
            nc.sync.dma_start(out=xg_sb[:, :], in_=xg[:, :])
            eblk = []
            for j in range(NT):
                sl = xg_sb[:, j * G:(j + 1) * G]
                nc.scalar.activation(out=sl, in_=sl,
                                     func=mybir.ActivationFunctionType.Exp,
                                     bias=bias_sb[:, :],
                                     accum_out=Z[:, j:j + 1])
                ga = gap.tile([128, 64], F32)
                nc.gpsimd.ap_gather(
                    out_ap=ga[:, :].rearrange("p (n d) -> p n d", d=1),
                    in_ap=sl.rearrange("p (n d) -> p n d", d=1),
                    idxs_ap=gidx_sb[:, :],
                    channels=128, num_elems=G, d=1, num_idxs=64,
                )
                eb = small.tile([BL, 8 * W], F32, tag=f"eb{j}")
                ebv = eb[:, :].rearrange("b (t s) -> b t s", s=W)
                # parity reshuffles: even partitions -> rows 0..7 (examples
                # 2r), odd -> rows 8..15 (examples 2(r-8)+1)
                nc.gpsimd.dma_start(out=ebv[0:8, :, :], in_=ga[0:128:2, 0:W])
                nc.gpsimd.dma_start(out=ebv[8:16, :, :],
                                    in_=ga[1:128:2, 32:32 + W])
                eblk.append(eb)

            # big stream: Z contributions only
            xrv = xr.rearrange("(n p) c -> n p c", p=128)
            for j in range(NT):
                xt = xio.tile([128, CR], F32)
                if j < NT - 1:
                    nc.sync.dma_start(out=xt[:, :], in_=xrv[j, :, :])
                    nc.scalar.activation(out=xt[:, :], in_=xt[:, :],
                                         func=mybir.ActivationFunctionType.Exp,
                                         bias=bias_sb[:, :],
                                         accum_out=Z[:, NT + j:NT + j + 1])
                else:
                    for h in range(4):
                        c0, c1 = LAST_CUTS[h], LAST_CUTS[h + 1]
                        nc.sync.dma_start(out=xt[:, c0:c1],
                                          in_=xrv[j, :, c0:c1])
                        nc.scalar.activation(
                            out=xt[:, c0:c1], in_=xt[:, c0:c1],
                            func=mybir.ActivationFunctionType.Exp,
                            bias=bias_sb[:, :],
                            accum_out=Z[:, NT + j + h:NT + j + h + 1])

            # ---- CTC forward DP, even/odd split, u-substitution ----
            u0 = small.tile([BL, W], F32, tag="u0")
            u1 = small.tile([BL, W], F32, tag="u1")
            O0 = small.tile([BL, W], F32, tag="O0")
            O1 = small.tile([BL, W], F32, tag="O1")
            ub = [u0, u1]
            Ob = [O0, O1]
            Sbuf = small.tile([BL, 5], F32)
            rec = small.tile([BL, 1], F32)
            ebS = small.tile([BL, 1], F32)
            nc.vector.memset(ub[0][:, :], 0.0)
            nc.vector.memset(Ob[0][:, :], 0.0)
            nc.vector.memset(Ob[1][:, 0:1], 0.0)
            # init (t=0): u0 = [eb0, eO0[0], 0...], O0 = [g, eO0[0], 0...]
            nc.vector.tensor_copy(out=ub[0][:, 0:2], in_=eblk[0][:, 0:2])
            nc.vector.tensor_copy(out=Ob[0][:, 1:2], in_=eblk[0][:, 1:2])

            k = 0
            for t in range(1, T):
                jj, tf = t // 8, t % 8
                base = tf * W
                ebt = eblk[jj][:, base:base + 1]
                eOt = eblk[jj][:, base + 1:base + W]
                uc, un = ub[(t - 1) % 2], ub[t % 2]
                Oc, On = Ob[(t - 1) % 2], Ob[t % 2]
                rdiv = RENORM_DIV.get(t)
                # O' = ((O + u) [*1/S]) * eO
                nc.vector.tensor_add(out=On[:, 1:W], in0=Oc[:, 1:W],
                                     in1=uc[:, 0:W - 1])
                if rdiv is None:
                    nc.vector.tensor_mul(out=On[:, 1:W], in0=On[:, 1:W],
                                         in1=eOt)
                    eb_use = ebt
                else:
                    nc.vector.reciprocal(out=rec[:, :],
                                         in_=Sbuf[:, rdiv:rdiv + 1])
                    nc.vector.scalar_tensor_tensor(
                        out=On[:, 1:W], in0=On[:, 1:W], scalar=rec[:, :],
                        in1=eOt, op0=mybir.AluOpType.mult,
                        op1=mybir.AluOpType.mult)
                    nc.vector.tensor_tensor(out=ebS[:, :], in0=ebt,
                                            in1=rec[:, :],
                                            op=mybir.AluOpType.mult)
                    eb_use = ebS[:, :]
                # u' = u * eb + shift(O'); renorm sum S = sum(u') for free
                acc = None
                if t in RENORM_ACC:
                    acc = Sbuf[:, k:k + 1]
                    k += 1
                nc.vector.scalar_tensor_tensor(
                    out=un[:, :], in0=uc[:, :], scalar=eb_use,
                    in1=On[:, 0:W], op0=mybir.AluOpType.mult,
                    op1=mybir.AluOpType.add, accum_out=acc)
            assert k == 5

            # final: export u_159 / S_159 (normalized), S slots, raw Z
            uf = small.tile([BL, W], F32)
            nc.vector.reciprocal(out=rec[:, :], in_=Sbuf[:, 4:5])
            nc.vector.tensor_scalar_mul(out=uf[:, :], in0=ub[(T - 1) % 2][:, :],
                                        scalar1=rec[:, :])
            nc.sync.dma_start(out=u_out[:, :], in_=uf[:, :])
            nc.sync.dma_start(out=s_out[:, :], in_=Sbuf[:, :])
            # z_out from the scalar queue: follows the last accum read on
            # the same engine, no cross-engine semaphore hop on the tail
            nc.scalar.dma_start(out=z_out[:, :], in_=Z[:, :])

    nc.compile()
    return nc


def _prep_core(predicts, labels, b0):
    """Host-side shard prep for examples [b0, b0+BL)."""
    xs = np.asarray(predicts[b0:b0 + BL])
    lab = labels[b0:b0 + BL].astype(np.int64)            # [BL, L]
    # class permutation: needed classes (blank + labels) first, pad to G
    needed = np.unique(np.concatenate([[0], lab.ravel()]))
    rest = np.setdiff1d(np.arange(C, dtype=np.int64), needed,
                        assume_unique=True)
    gcols = np.concatenate([needed, rest[:G - len(needed)]])
    rcols = rest[G - len(needed):]
    pos_of = np.zeros(C, np.int64)
    pos_of[gcols] = np.arange(G)

    # row permutation: tile j, partition p = 16g + 2tf + c -> (b=2g+c, 8j+tf)
    def rows(a, width):
        return np.ascontiguousarray(
            a.reshape(8, 2, NT, 8, width).transpose(2, 0, 3, 1, 4)
            .reshape(NT * 128, width))

    xg = rows(np.ascontiguousarray(xs[:, :, gcols], dtype=np.float32), G)
    xg = np.ascontiguousarray(
        xg.reshape(NT, 128, G).transpose(1, 0, 2).reshape(128, NT * G))
    xr = rows(np.ascontiguousarray(xs[:, :, rcols], dtype=np.float32), CR)

    # union gather list per group g (= example pair 2g, 2g+1), positions
    # into the permuted class space
    lists = np.zeros((8, 64), np.int64)
    lists[:, 1:W] = pos_of[lab[0::2]]
    lists[:, 33:32 + W] = pos_of[lab[1::2]]
    # ap_gather wrap: index position pos = slot*16 + (p % 16)
    p_idx = np.arange(128)
    slot = np.arange(4)
    gidx = lists[(p_idx[:, None] // 16),
                 slot[None, :] * 16 + (p_idx[:, None] % 16)]
    return {"xg": xg, "xr": xr, "gidx": gidx.astype(np.int16)}


def _host_exact_loss(x_b, lab_b, len_b):
    """Float64 log-space CTC for one example (repeat-label fallback)."""
    xb = x_b.astype(np.float64)
    m = xb.max(axis=1, keepdims=True)
    lp = xb - (np.log(np.exp(xb - m).sum(axis=1, keepdims=True)) + m)
    l = int(len_b)
    labs = lab_b[:l].astype(int)
    S2 = 2 * l + 1
    ext = np.zeros(S2, int)
    ext[1::2] = labs
    allow = np.zeros(S2, bool)
    allow[3::2] = ext[3::2] != ext[1:S2 - 2:2]
    a = np.full(S2, -np.inf)
    a[0] = lp[0, 0]
    a[1] = lp[0, ext[1]]
    for t in range(1, T):
        sh1 = np.concatenate(([-np.inf], a[:-1]))
        sh2 = np.concatenate(([-np.inf, -np.inf], a[:-2]))
        sh2 = np.where(allow, sh2, -np.inf)
        mx = np.maximum(np.maximum(a, sh1), sh2)
        acc = mx + np.log(np.exp(a - mx) + np.exp(sh1 - mx) + np.exp(sh2 - mx))
        a = acc + lp[t, ext]
    return -np.logaddexp(a[2 * l], a[2 * l - 1])


_NC_CACHE = []


def kernel(predicts, labels, label_lengths):
    predicts = np.asarray(predicts)
    labels = np.asarray(labels)
    label_lengths = np.asarray(label_lengths)
    if not _NC_CACHE:
        _NC_CACHE.append(_build_kernel())
    nc = _NC_CACHE[0]
    in_maps = [_prep_core(predicts, labels, k * BL) for k in range(NCORES)]
    res = bass_utils.run_bass_kernel_spmd(nc, in_maps, core_ids=list(range(NCORES)))

    # host readout in float64
    lens = label_lengths.astype(np.int64)
    losses = np.zeros(B, np.float64)
    for kcore in range(NCORES):
        r = res.results[kcore]
        u = r["u_out"].reshape(BL, W).astype(np.float64)
        S = r["s_out"].reshape(BL, 5).astype(np.float64)
        Z = r["z_out"].reshape(128, NZ).astype(np.float64)
        # per-(p, tile) Z: xg col j + xr col; last tile = 3 chunks
        Zt = Z[:, 0:NT].copy()
        Zt[:, :NT - 1] += Z[:, NT:2 * NT - 1]
        Zt[:, NT - 1] += Z[:, 2 * NT - 1:].sum(axis=1)
        # p = 16g + 2tf + c -> example 2g+c, t = 8j+tf
        p = np.arange(128)
        b_of = 2 * (p // 16) + (p % 2)
        tf_of = (p % 16) // 2
        Zbt = np.zeros((BL, T))
        Zbt[b_of[:, None], 8 * np.arange(NT)[None, :] + tf_of[:, None]] = Zt
        slZ = np.log(Zbt).sum(axis=1)                      # [BL]
        # DP row r -> example 2r (r<8) or 2(r-8)+1
        lnS = np.log(S).sum(axis=1)                        # per row
        for r in range(BL):
            ex = 2 * r if r < 8 else 2 * (r - 8) + 1
            b = kcore * BL + ex
            v = u[r, lens[b]]
            losses[b] = slZ[ex] - (np.log(v) + lnS[r])

    # exact host recompute for examples with consecutive repeated labels
    for b in range(B):
        l = lens[b]
        if l >= 2 and (labels[b, 1:l] == labels[b, :l - 1]).any():
            losses[b] = _host_exact_loss(predicts[b], labels[b], l)

    w = np.square(1.0 - np.exp(-losses))
    return np.float32(np.mean(losses * w))


# revision 13
# speedup vs baseline: 1.0639x; 1.0639x over previous
"""CTC loss (focal-reweighted) Trainium2 Bass kernel, v3.

Data parallel over batch (128 examples -> 8 cores x 16). Per core:
  - host permutes the class axis: the <=401 classes that the CTC DP ever
    reads (blank + every label of the core's 16 examples) are packed into
    a 512-column tensor xg; the remaining 6113 columns form xr. Z (the
    softmax denominator) = rowsum exp(xg) + rowsum exp(xr), all on device.
  - xg ([128, 20*512], tile j = 16 examples x 8 timesteps, partition
    p = 16g + 2*tf + c -> example 2g+c, t = 8j+tf) loads in ONE DMA up
    front; 20 small exps (accum -> Z) feed 20 ap_gathers + parity
    reshuffles, so the CTC DP runs ~100us ahead of the big stream.
  - xr streams as 20 tiles of [128, 6113]; exp+accum -> Z columns. The
    last tile is split into 3 column chunks so the final exp (Z-only) is
    ~2us instead of ~6us after the stream ends.
  - CTC forward DP in prob space, even/odd state split: even (blank)
    states all share the per-example blank prob, so tracking
    u[i] = E[i] + O[i-1] and O gives 3 DVE ops per timestep:
      O' = (O + u) * eO;  u' = u * eb + shift(O')
    with renorms every 32 steps folded into the next step's multiplies
    and renorm sums taken from scalar_tensor_tensor's accum_out.
  - device exports raw u_T [16,26], S [16,5], Z [128,42]; host does all
    logs / readout / focal / mean in float64. v = u_T[len] directly.
Device DP assumes no consecutive repeated labels (skip always allowed on
odd states); host detects repeat examples and recomputes those exactly.
"""

import numpy as np

import concourse.bass as bass
import concourse.bacc as bacc
import concourse.tile as tile
from concourse import mybir
from concourse import bass_utils

B, T, C, L = 128, 160, 6625, 25
NCORES = 8
BL = B // NCORES          # 16 examples per core
NT = 20                   # t-blocks of 8 timesteps
G = 512                   # gathered-class columns (>= 401 needed)
CR = C - G                # 6113 remaining columns
W = L + 1                 # 26: [blank, 25 labels] per example per t
NZ = NT + (NT - 1) + 3    # Z accum columns: 20 xg + 19 xr + 3 last-tile
RENORM_ACC = (32, 64, 96, 128, 159)
RENORM_DIV = {33: 0, 65: 1, 97: 2, 129: 3}
LAST_CUTS = (0, 2038, 4076, CR)

F32 = mybir.dt.float32
I16 = mybir.dt.int16
LN2 = 0.6931471805599453
EXP_BIAS = -2.0 * LN2     # exp(x - 2 ln2) = 0.25 e^x, keeps DP in fp32 range


def _build_kernel():
    nc = bacc.Bacc("TRN2", target_bir_lowering=False, debug=False)
    xg = nc.dram_tensor("xg", [128, NT * G], F32, kind="ExternalInput").ap()
    xr = nc.dram_tensor("xr", [BL * T, CR], F32, kind="ExternalInput").ap()
    gidx = nc.dram_tensor("gidx", [128, 4], I16, kind="ExternalInput").ap()
    u_out = nc.dram_tensor("u_out", [BL, W], F32, kind="ExternalOutput").ap()
    s_out = nc.dram_tensor("s_out", [BL, 5], F32, kind="ExternalOutput").ap()
    z_out = nc.dram_tensor("z_out", [128, NZ], F32, kind="ExternalOutput").ap()

    with tile.TileContext(nc) as tc:
        with (
            tc.tile_pool(name="xio", bufs=5) as xio,
            tc.tile_pool(name="gap", bufs=4) as gap,
            tc.tile_pool(name="small", bufs=1) as small,
        ):
            gidx_sb = small.tile([128, 4], I16)
            nc.sync.dma_start(out=gidx_sb[:, :], in_=gidx[:, :])
            bias_sb = small.tile([128, 1], F32)
            nc.vector.memset(bias_sb[:, :], EXP_BIAS)

            Z = small.tile([128, NZ], F32)
            # gathered-class columns: one up-front DMA, then small exps,
            # gathers and reshuffles run far ahead of the big stream
            xg_sb = small.tile([128, NT * G], F32)
            nc.sync.dma_start(out=xg_sb[:, :], in_=xg[:, :])
            eblk = []
            for j in range(NT):
                sl = xg_sb[:, j * G:(j + 1) * G]
                nc.scalar.activation(out=sl, in_=sl,
                                     func=mybir.ActivationFunctionType.Exp,
                                     bias=bias_sb[:, :],
                                     accum_out=Z[:, j:j + 1])
                ga = gap.tile([128, 64], F32)
                nc.gpsimd.ap_gather(
                    out_ap=ga[:, :].rearrange("p (n d) -> p n d", d=1),
                    in_ap=sl.rearrange("p (n d) -> p n d", d=1),
                    idxs_ap=gidx_sb[:, :],
                    channels=128, num_elems=G, d=1, num_idxs=64,
                )
                eb = small.tile([BL, 8 * W], F32, tag=f"eb{j}")
                ebv = eb[:, :].rearrange("b (t s) -> b t s", s=W)
                # parity reshuffles: even partitions -> rows 0..7 (examples
                # 2r), odd -> rows 8..15 (examples 2(r-8)+1)
                nc.gpsimd.dma_start(out=ebv[0:8, :, :], in_=ga[0:128:2, 0:W])
                nc.gpsimd.dma_start(out=ebv[8:16, :, :],
                                    in_=ga[1:128:2, 32:32 + W])
                eblk.append(eb)

            # big stream: Z contributions only
            xrv = xr.rearrange("(n p) c -> n p c", p=128)
            for j in range(NT):
                xt = xio.tile([128, CR], F32)
                if j < NT - 1:
                    nc.sync.dma_start(out=xt[:, :], in_=xrv[j, :, :])
                    nc.scalar.activation(out=xt[:, :], in_=xt[:, :],
                                         func=mybir.ActivationFunctionType.Exp,
                                         bias=bias_sb[:, :],
                                         accum_out=Z[:, NT + j:NT + j + 1])
                else:
                    for h in range(3):
                        c0, c1 = LAST_CUTS[h], LAST_CUTS[h + 1]
                        nc.sync.dma_start(out=xt[:, c0:c1],
                                          in_=xrv[j, :, c0:c1])
                        nc.scalar.activation(
                            out=xt[:, c0:c1], in_=xt[:, c0:c1],
                            func=mybir.ActivationFunctionType.Exp,
                            bias=bias_sb[:, :],
                            accum_out=Z[:, NT + j + h:NT + j + h + 1])

            # ---- CTC forward DP, even/odd split, u-substitution ----
            u0 = small.tile([BL, W], F32, tag="u0")
            u1 = small.tile([BL, W], F32, tag="u1")
            O0 = small.tile([BL, W], F32, tag="O0")
            O1 = small.tile([BL, W], F32, tag="O1")
            ub = [u0, u1]
            Ob = [O0, O1]
            Sbuf = small.tile([BL, 5], F32)
            rec = small.tile([BL, 1], F32)
            ebS = small.tile([BL, 1], F32)
            nc.vector.memset(ub[0][:, :], 0.0)
            nc.vector.memset(Ob[0][:, :], 0.0)
            nc.vector.memset(Ob[1][:, 0:1], 0.0)
            # init (t=0): u0 = [eb0, eO0[0], 0...], O0 = [g, eO0[0], 0...]
            nc.vector.tensor_copy(out=ub[0][:, 0:2], in_=eblk[0][:, 0:2])
            nc.vector.tensor_copy(out=Ob[0][:, 1:2], in_=eblk[0][:, 1:2])

            k = 0
            for t in range(1, T):
                jj, tf = t // 8, t % 8
                base = tf * W
                ebt = eblk[jj][:, base:base + 1]
                eOt = eblk[jj][:, base + 1:base + W]
                uc, un = ub[(t - 1) % 2], ub[t % 2]
                Oc, On = Ob[(t - 1) % 2], Ob[t % 2]
                rdiv = RENORM_DIV.get(t)
                # O' = ((O + u) [*1/S]) * eO
                nc.vector.tensor_add(out=On[:, 1:W], in0=Oc[:, 1:W],
                                     in1=uc[:, 0:W - 1])
                if rdiv is None:
                    nc.vector.tensor_mul(out=On[:, 1:W], in0=On[:, 1:W],
                                         in1=eOt)
                    eb_use = ebt
                else:
                    nc.vector.reciprocal(out=rec[:, :],
                                         in_=Sbuf[:, rdiv:rdiv + 1])
                    nc.vector.scalar_tensor_tensor(
                        out=On[:, 1:W], in0=On[:, 1:W], scalar=rec[:, :],
                        in1=eOt, op0=mybir.AluOpType.mult,
                        op1=mybir.AluOpType.mult)
                    nc.vector.tensor_tensor(out=ebS[:, :], in0=ebt,
                                            in1=rec[:, :],
                                            op=mybir.AluOpType.mult)
                    eb_use = ebS[:, :]
                # u' = u * eb + shift(O'); renorm sum S = sum(u') for free
                acc = None
                if t in RENORM_ACC:
                    acc = Sbuf[:, k:k + 1]
                    k += 1
                nc.vector.scalar_tensor_tensor(
                    out=un[:, :], in0=uc[:, :], scalar=eb_use,
                    in1=On[:, 0:W], op0=mybir.AluOpType.mult,
                    op1=mybir.AluOpType.add, accum_out=acc)
            assert k == 5

            # final: export u_159 / S_159 (normalized), S slots, raw Z
            uf = small.tile([BL, W], F32)
            nc.vector.reciprocal(out=rec[:, :], in_=Sbuf[:, 4:5])
            nc.vector.tensor_scalar_mul(out=uf[:, :], in0=ub[(T - 1) % 2][:, :],
                                        scalar1=rec[:, :])
            nc.sync.dma_start(out=u_out[:, :], in_=uf[:, :])
            nc.sync.dma_start(out=s_out[:, :], in_=Sbuf[:, :])
            nc.sync.dma_start(out=z_out[:, :], in_=Z[:, :])

    nc.compile()
    return nc


def _prep_core(predicts, labels, b0):
    """Host-side shard prep for examples [b0, b0+BL)."""
    xs = np.asarray(predicts[b0:b0 + BL])
    lab = labels[b0:b0 + BL].astype(np.int64)            # [BL, L]
    # class permutation: needed classes (blank + labels) first, pad to G
    needed = np.unique(np.concatenate([[0], lab.ravel()]))
    rest = np.setdiff1d(np.arange(C, dtype=np.int64), needed,
                        assume_unique=True)
    gcols = np.concatenate([needed, rest[:G - len(needed)]])
    rcols = rest[G - len(needed):]
    pos_of = np.zeros(C, np.int64)
    pos_of[gcols] = np.arange(G)

    # row permutation: tile j, partition p = 16g + 2tf + c -> (b=2g+c, 8j+tf)
    def rows(a, width):
        return np.ascontiguousarray(
            a.reshape(8, 2, NT, 8, width).transpose(2, 0, 3, 1, 4)
            .reshape(NT * 128, width))

    xg = rows(np.ascontiguousarray(xs[:, :, gcols], dtype=np.float32), G)
    xg = np.ascontiguousarray(
        xg.reshape(NT, 128, G).transpose(1, 0, 2).reshape(128, NT * G))
    xr = rows(np.ascontiguousarray(xs[:, :, rcols], dtype=np.float32), CR)

    # union gather list per group g (= example pair 2g, 2g+1), positions
    # into the permuted class space
    lists = np.zeros((8, 64), np.int64)
    lists[:, 1:W] = pos_of[lab[0::2]]
    lists[:, 33:32 + W] = pos_of[lab[1::2]]
    # ap_gather wrap: index position pos = slot*16 + (p % 16)
    p_idx = np.arange(128)
    slot = np.arange(4)
    gidx = lists[(p_idx[:, None] // 16),
                 slot[None, :] * 16 + (p_idx[:, None] % 16)]
    return {"xg": xg, "xr": xr, "gidx": gidx.astype(np.int16)}


def _host_exact_loss(x_b, lab_b, len_b):
    """Float64 log-space CTC for one example (repeat-label fallback)."""
    xb = x_b.astype(np.float64)
    m = xb.max(axis=1, keepdims=True)
    lp = xb - (np.log(np.exp(xb - m).sum(axis=1, keepdims=True)) + m)
    l = int(len_b)
    labs = lab_b[:l].astype(int)
    S2 = 2 * l + 1
    ext = np.zeros(S2, int)
    ext[1::2] = labs
    allow = np.zeros(S2, bool)
    allow[3::2] = ext[3::2] != ext[1:S2 - 2:2]
    a = np.full(S2, -np.inf)
    a[0] = lp[0, 0]
    a[1] = lp[0, ext[1]]
    for t in range(1, T):
        sh1 = np.concatenate(([-np.inf], a[:-1]))
        sh2 = np.concatenate(([-np.inf, -np.inf], a[:-2]))
        sh2 = np.where(allow, sh2, -np.inf)
        mx = np.maximum(np.maximum(a, sh1), sh2)
        acc = mx + np.log(np.exp(a - mx) + np.exp(sh1 - mx) + np.exp(sh2 - mx))
        a = acc + lp[t, ext]
    return -np.logaddexp(a[2 * l], a[2 * l - 1])


_NC_CACHE = []


def kernel(predicts, labels, label_lengths):
    predicts = np.asarray(predicts)
    labels = np.asarray(labels)
    label_lengths = np.asarray(label_lengths)
    if not _NC_CACHE:
        _NC_CACHE.append(_build_kernel())
    nc = _NC_CACHE[0]
    in_maps = [_prep_core(predicts, labels, k * BL) for k in range(NCORES)]
    res = bass_utils.run_bass_kernel_spmd(nc, in_maps, core_ids=list(range(NCORES)))

    # host readout in float64
    lens = label_lengths.astype(np.int64)
    losses = np.zeros(B, np.float64)
    for kcore in range(NCORES):
        r = res.results[kcore]
        u = r["u_out"].reshape(BL, W).astype(np.float64)
        S = r["s_out"].reshape(BL, 5).astype(np.float64)
        Z = r["z_out"].reshape(128, NZ).astype(np.float64)
        # per-(p, tile) Z: xg col j + xr col; last tile = 3 chunks
        Zt = Z[:, 0:NT].copy()
        Zt[:, :NT - 1] += Z[:, NT:2 * NT - 1]
        Zt[:, NT - 1] += Z[:, 2 * NT - 1:].sum(axis=1)
        # p = 16g + 2tf + c -> example 2g+c, t = 8j+tf
        p = np.arange(128)
        b_of = 2 * (p // 16) + (p % 2)
        tf_of = (p % 16) // 2
        Zbt = np.zeros((BL, T))
        Zbt[b_of[:, None], 8 * np.arange(NT)[None, :] + tf_of[:, None]] = Zt
        slZ = np.log(Zbt).sum(axis=1)                      # [BL]
        # DP row r -> example 2r (r<8) or 2(r-8)+1
        lnS = np.log(S).sum(axis=1)                        # per row
        for r in range(BL):
            ex = 2 * r if r < 8 else 2 * (r - 8) + 1
            b = kcore * BL + ex
            v = u[r, lens[b]]
            losses[b] = slZ[ex] - (np.log(v) + lnS[r])

    # exact host recompute for examples with consecutive repeated labels
    for b in range(B):
        l = lens[b]
        if l >= 2 and (labels[b, 1:l] == labels[b, :l - 1]).any():
            losses[b] = _host_exact_loss(predicts[b], labels[b], l)

    w = np.square(1.0 - np.exp(-losses))
    return np.float32(np.mean(losses * w))
